# revision 80
# baseline (speedup 1.0000x reference)
"""MultiHeadAttention Trainium2 kernel (8 NeuronCores).

Sharding: core c -> (batch b = c//2, head-group g = c%2) of the 12 heads.
Each core computes attention for its 6 heads of one batch element and a
partial projection; the host sums the two head-group partials per batch
element and adds the effective proj bias (proj_b + bv @ proj_w; the v bias
is additive after softmax because attention rows sum to 1).

Per-core dataflow:
  x8/xr8 fp8 (value + residual) DMA'd pre-transposed (host prep does the
    transpose, the 64x weight scaling, and the fp8 residual splits)
  q/k psum f32 = 3-term fp8 DoubleRow matmul (x8@w8 + xr8@w8 + x8@wr8,
    error ~ fp8_eps^2); merged [128,512] drains add bias and convert to
    the fp8 zero-padded DoubleRow layout q8/k8 [128,2,S]
  v = 3-term fp8 DR likewise; bf16 seq-major vsl [skpair, j, head, 65]
    (+ ones col for denominator)
  scores[sk,sq] = DoubleRow fp8 matmul (contraction 64 + 64 zeros)
  pt = exp(scores/8): split ACT (exact, ~10/16 units) / DVE (Schraudolph
    bits at 6/16; 8:8 in the last two blocks to shorten the tail), bf16
  av[sq,2,65] += pt-chunk^T @ v    (bf16, psum accum; col 64 = denom)
  ao_n = av * 1/denom (DVE), DMA-XBAR transpose into aoT
  yT[768,S] bf16 = (wp/64)^T @ aoT (partial projection, host sums pairs)

Schedule: per 512-query block, 16 score units stream to the exp engines
while PE filler thunks (next-pair q/k, v, prev-block AV via hooks, proj
for pair-2 blocks) keep the PE busy; psum tiles allocate lazily inside
thunks so the 3-deep f32 score ring stays available; the PE-bound v-lump
block (0,1) additionally absorbs the first half of block (0,2)'s score
units to feed the otherwise-idle exp engines; the tail runs the
second-to-last proj chunk-split while the last exps drain.
"""
import sys

sys.path.insert(0, "/opt/trn_rl_repo")

import numpy as np

import concourse.bass as bass
import concourse.mybir as mybir
import concourse.tile as tile
from concourse import bacc
from concourse.bass_utils import run_bass_kernel_spmd

F32 = mybir.dt.float32
BF16 = mybir.dt.bfloat16
U16 = mybir.dt.uint16
FP8 = mybir.dt.float8e4
EXP = mybir.ActivationFunctionType.Exp
COPY_FN = mybir.ActivationFunctionType.Identity
ADD = mybir.AluOpType.add
MULT = mybir.AluOpType.mult
DIV = mybir.AluOpType.divide
DR = mybir.MatmulPerfMode.DoubleRow

HID = 768
D = 64  # head dim
LHEADS = 6  # heads per core
PAIRS = 3

LOG2E = 1.4426950408889634
# Schraudolph-in-bf16-bits: n = x*0.125*128*log2e + (16256 + c); floor via
# the executor's f32->u16 cast. c=-6.85 calibrated for min rms vs exp().
WSCALE = 64.0  # host scales wqk/wv by 64 (fp8 denormal avoidance)
SCHR_MUL = 0.125 * 128.0 * LOG2E / (WSCALE * WSCALE)
SCHR_ADD = 16256.0 - 6.85
EXP_SCALE = 0.125 / (WSCALE * WSCALE)

# unit u (0..15) -> exp engine, alternating 10:6 / 9:7 ACT:DVE per block;
# the final block runs 8:8 so the exp drain into the tail is shortest
ACT_UNIT = (
    tuple(u % 8 in (0, 1, 3, 4, 6) for u in range(16)),
    tuple(u % 8 in (0, 1, 3, 4, 6) for u in range(8))
    + tuple(u % 8 in (0, 2, 4, 6) for u in range(8)),
    tuple(u % 2 == 0 for u in range(16)),
)


def build_nc(S: int):
    nc = bacc.Bacc("TRN2", target_bir_lowering=False, debug=False)
    NSEQ = S // 128  # seq chunks of 128
    NBLK = S // 512  # seq blocks of 512
    NPAIR = NSEQ // 2  # sk chunk pairs

    x8d = nc.dram_tensor("x8", [HID, S], FP8, kind="ExternalInput")
    xr8d = nc.dram_tensor("xr8", [HID, S], FP8, kind="ExternalInput")
    w8d = nc.dram_tensor("w8", [HID, 768], FP8, kind="ExternalInput")
    wr8d = nc.dram_tensor("wr8", [HID, 768], FP8, kind="ExternalInput")
    bqk = nc.dram_tensor("bqk", [768], F32, kind="ExternalInput")
    wv8d = nc.dram_tensor("wv8", [HID, 384], FP8, kind="ExternalInput")
    wvr8d = nc.dram_tensor("wvr8", [HID, 384], FP8, kind="ExternalInput")
    wp = nc.dram_tensor("wp", [384, HID], BF16, kind="ExternalInput")
    yT = nc.dram_tensor("yT", [HID, S], BF16, kind="ExternalOutput")

    with tile.TileContext(nc) as tc:
        with (
            tc.tile_pool(name="const", bufs=1) as cp,
            tc.tile_pool(name="wts", bufs=1) as wpool,
            tc.tile_pool(name="xs", bufs=1) as xsp,
            tc.tile_pool(name="qk8", bufs=1) as qk8p,
            tc.tile_pool(name="vv", bufs=1) as vvp,
            tc.tile_pool(name="ao", bufs=1) as aop,
            tc.tile_pool(name="ps", bufs=1, space="PSUM") as ps,
        ):
            # load the exp ACT table off the critical path
            warm = cp.tile([1, 16], F32, tag="warm")
            nc.vector.memset(warm[:], 1.0)
            nc.scalar.activation(warm[:], warm[:], EXP, bias=0.0, scale=0.0)

            # fp8 q/k, zero-padded DoubleRow layout, one tile per pair:
            # [128, 2, S]; partition half = head-of-pair, dim1 = j (j=1
            # stays zero, via gpsimd memset off the DMA path) so
            # [64*hi:64*hi+64, :, a:b] is a DR operand.
            q8_tiles = {}
            k8_tiles = {}

            def get_qk8(p):
                if p not in q8_tiles:
                    q8_p = qk8p.tile([128, 2, S], FP8, tag="q8", bufs=3,
                                     name=f"q8_{p}")
                    k8_p = qk8p.tile([128, 2, S], FP8, tag="k8", bufs=3,
                                     name=f"k8_{p}")
                    nc.gpsimd.memset(q8_p[:, 1, :], 0.0)
                    nc.gpsimd.memset(k8_p[:, 1, :], 0.0)
                    q8_tiles[p] = q8_p
                    k8_tiles[p] = k8_p
                return q8_tiles[p], k8_tiles[p]

            for _p in range(PAIRS):
                get_qk8(_p)

            # weights: wqk/wv fp8 (hidden-chunk-major for DR), wp bf16;
            # DMA order tuned so the first qk matmul unblocks earliest
            wqk8_r = wpool.tile([128, 6, 768], FP8, tag="wqk8")
            nc.sync.dma_start(
                wqk8_r[:], w8d[:].rearrange("(c p) f -> p c f", p=128)
            )
            xT8 = xsp.tile([128, 6, S], FP8, tag="xT8")
            xTr8 = xsp.tile([128, 6, S], FP8, tag="xTr8")
            x8_ap = x8d[:].rearrange("(c p) s -> p c s", p=128)
            xr8_ap = xr8d[:].rearrange("(c p) s -> p c s", p=128)
            nc.sync.dma_start(xT8[:, :, 0:512], x8_ap[:, :, 0:512])
            nc.sync.dma_start(xTr8[:, :, 0:512], xr8_ap[:, :, 0:512])
            wqkr8_r = wpool.tile([128, 6, 768], FP8, tag="wqkr8")
            nc.sync.dma_start(
                wqkr8_r[:], wr8d[:].rearrange("(c p) f -> p c f", p=128)
            )
            bqk_sb = cp.tile([128, 6], F32, tag="bqk")
            nc.sync.dma_start(bqk_sb[:], bqk[:].rearrange("(c p) -> p c", p=128))
            for g in range(1, NBLK):
                sl = slice(g * 512, (g + 1) * 512)
                nc.sync.dma_start(xT8[:, :, sl], x8_ap[:, :, sl])
                nc.sync.dma_start(xTr8[:, :, sl], xr8_ap[:, :, sl])
            wv8_r = wpool.tile([128, 6, 384], FP8, tag="wv8")
            nc.sync.dma_start(
                wv8_r[:], wv8d[:].rearrange("(c p) f -> p c f", p=128)
            )
            wvr8_r = wpool.tile([128, 6, 384], FP8, tag="wvr8")
            nc.sync.dma_start(
                wvr8_r[:], wvr8d[:].rearrange("(c p) f -> p c f", p=128)
            )
            wp_r = wpool.tile([128, 3, HID], BF16, tag="wpr")
            nc.sync.dma_start(
                wp_r[:], wp[:].rearrange("(c p) f -> p c f", p=128)
            )

            # v seq-major [sk, skpair, j, head, 65]; col 64 = ones
            vsl = vvp.tile([128, NPAIR, 2, LHEADS, D + 1], BF16, tag="v")
            nc.gpsimd.memset(vsl[:, :, :, :, D : D + 1], 1.0)

            aoT = aop.tile([128, PAIRS, S], BF16, tag="aoT")
            yT_ap = yT[:].rearrange("(c p) s -> p c s", p=128)

            with (
                tc.tile_pool(name="pt", bufs=1) as ptp,
                tc.tile_pool(name="sm", bufs=1) as smp,
            ):

                def emit_qk_fillers(p, n):
                    """q+k for pair p block n (3-term fp8 DR: x8@w8 +
                    xr8@w8 + x8@wr8, error ~ fp8_eps^2) as thunks that
                    interleave between score units. The psum tile is
                    allocated lazily by the first thunk so it doesn't
                    pin an sc-ring slot for the whole block."""
                    box = {}

                    def get_qp():
                        if "qp" not in box:
                            box["qp"] = ps.tile([128, 2, 512], F32,
                                                tag="sc", bufs=3,
                                                name=f"qp_{p}_{n}")
                        return box["qp"]

                    terms = ((xT8, wqk8_r), (xTr8, wqk8_r), (xT8, wqkr8_r))

                    def mk_mm(qk_i, wcol, t, c):
                        xt, wt = terms[t]

                        def mm():
                            nc.tensor.matmul(
                                get_qp()[:, qk_i, :],
                                wt[:, 2 * c : 2 * c + 2,
                                   wcol * 128 : (wcol + 1) * 128],
                                xt[:, 2 * c : 2 * c + 2,
                                   n * 512 : (n + 1) * 512],
                                start=(t == 0 and c == 0),
                                stop=(t == 2 and c == 2),
                                perf_mode=DR,
                                skip_group_check=True,
                            )
                        return mm

                    fillers = []
                    for qk_i, wcol in ((0, p), (1, 3 + p)):
                        for t in range(3):
                            for c in range(3):
                                fillers.append(mk_mm(qk_i, wcol, t, c))

                    def drains():
                        sl = slice(n * 512, (n + 1) * 512)
                        q8_p, k8_p = get_qk8(p)
                        qp = get_qp()
                        nc.vector.tensor_scalar(
                            q8_p[:, 0, sl], qp[:, 0, :],
                            bqk_sb[:, p : p + 1], None, ADD,
                        )
                        nc.scalar.activation(
                            k8_p[:, 0, sl], qp[:, 1, :], COPY_FN,
                            bias=bqk_sb[:, 3 + p : 4 + p], scale=1.0,
                        )
                    fillers.append(drains)
                    return fillers

                def emit_qk(p, n):
                    for f in emit_qk_fillers(p, n):
                        f()

                def emit_v(ip):
                    """v for seq chunk pair ip (chunks 2ip, 2ip+1), all 6
                    heads, seq-major, bias-free. 3-term fp8 DR:
                    x8@wv8 + xr8@wv8 + x8@wvr8 (error ~ fp8_eps^2)."""
                    vp = ps.tile([128, 2, 512], F32, tag="sc", bufs=3)
                    terms = ((xT8, wv8_r), (xTr8, wv8_r), (xT8, wvr8_r))
                    for j in range(2):
                        i = 2 * ip + j
                        for t, (xt, wt) in enumerate(terms):
                            for c in range(3):
                                nc.tensor.matmul(
                                    vp[:, j, 0:384],
                                    xt[:, 2 * c : 2 * c + 2,
                                       i * 128 : (i + 1) * 128],
                                    wt[:, 2 * c : 2 * c + 2, :],
                                    start=(t == 0 and c == 0),
                                    stop=(t == 2 and c == 2),
                                    perf_mode=DR,
                                    skip_group_check=True,
                                )
                    nc.vector.tensor_copy(
                        vsl[:, ip, :, :, 0:D],
                        vp[:, :, 0:384].rearrange("p j (h d) -> p j h d", h=6),
                    )

                alt_pt = {}

                def emit_score_unit(p, n, pt, hi, i, act=None, dst=None):
                    blk = p * NBLK + n
                    q8_p, k8_p = get_qk8(p)
                    sc = ps.tile([128, 2, 512], F32, tag="sc", bufs=3,
                                 name="sc")
                    for j in range(2):
                        sk = 2 * i + j
                        nc.tensor.matmul(
                            sc[:, j, :],
                            k8_p[64 * hi : 64 * hi + 64, :,
                                 sk * 128 : (sk + 1) * 128],
                            q8_p[64 * hi : 64 * hi + 64, :,
                                 n * 512 : (n + 1) * 512],
                            start=True,
                            stop=True,
                            perf_mode=DR,
                        )
                    u = 8 * hi + i
                    if dst is None:
                        dst = pt[:, u, :, :]
                    pat = 2 if blk >= PAIRS * NBLK - 2 else blk % 2
                    if ACT_UNIT[pat][u] if act is None else act:
                        nc.scalar.activation(
                            dst, sc[:], EXP, bias=0.0, scale=EXP_SCALE
                        )
                    else:
                        nc.vector.tensor_scalar(
                            dst.bitcast(U16),
                            sc[:],
                            SCHR_MUL, SCHR_ADD, MULT, ADD,
                        )

                def emit_scores(p, n, pt, hook=None, fillers=None,
                                hook_at=(3, 7, 11, 14)):
                    fillers = list(fillers or [])
                    u = 0
                    for hi in range(2):
                        for i in range(NPAIR):
                            emit_score_unit(p, n, pt, hi, i)
                            # dispatch filler PE work between units so sc
                            # tiles keep flowing to the exp engines
                            units_left = 16 - u
                            share = -(-len(fillers) // units_left)
                            for _ in range(share):
                                fillers.pop(0)()
                            if hook is not None and u in hook_at:
                                hook(hook_at.index(u))
                            u += 1
                    for f in fillers:
                        f()

                def emit_av_half(p, n, pt, c, hi, av):
                    for i in range(NPAIR):
                        src = pt[:, 8 * hi + i]
                        if (p, n, hi, i) in alt_pt:
                            t, slot = alt_pt[(p, n, hi, i)]
                            src = t[:, slot]
                        for j in range(2):
                            nc.tensor.matmul(
                                av[:, hi, :],
                                src[:, j, c * 128 : (c + 1) * 128],
                                vsl[:, i, j, 2 * p + hi, :],
                                start=(hi == 0 and i == 0 and j == 0),
                                stop=(hi == 1 and i == NPAIR - 1
                                      and j == 1),
                                skip_group_check=True,
                            )

                def emit_av_fin(p, n, c, av):
                    rec = smp.tile([128, 2], F32, tag="rec", bufs=4)
                    nc.vector.reciprocal(rec[:], av[:, :, D])

                    ao_n = smp.tile([128, 2, D], BF16, tag="aon", bufs=16,
                                    name=f"ao_n_{p}_{n}_{c}")
                    nc.vector.tensor_tensor(
                        ao_n[:],
                        av[:, :, 0:D],
                        rec[:].unsqueeze(2).broadcast_to([128, 2, D]),
                        MULT,
                    )
                    # XBAR transpose [q, (hi d)] -> [(hi d), q] into aoT
                    nc.sync.dma_start(
                        aoT[:, p, n * 512 + c * 128 : n * 512 + (c + 1) * 128],
                        ao_n[:].rearrange("p h d -> p (h d)"),
                        transpose=True,
                    )

                def emit_av_chunk(p, n, pt, c):
                    av = ps.tile([128, 2, D + 1], F32, tag="av", bufs=2)
                    emit_av_half(p, n, pt, c, 0, av)
                    emit_av_half(p, n, pt, c, 1, av)
                    emit_av_fin(p, n, c, av)

                def emit_av_fillers(p, n, pt):
                    out = []
                    for c in range(4):
                        box = {}

                        def t1(c=c, box=box):
                            box["av"] = ps.tile([128, 2, D + 1], F32,
                                                tag="av", bufs=2,
                                                name=f"av_{p}_{n}_{c}")
                            emit_av_half(p, n, pt, c, 0, box["av"])

                        def t2(c=c, box=box):
                            emit_av_half(p, n, pt, c, 1, box["av"])

                        def t3(c=c, box=box):
                            emit_av_fin(p, n, c, box["av"])

                        out += [t1, t2, t3]
                    return out

                def emit_av(p, n, pt):
                    for c in range(4):
                        emit_av_chunk(p, n, pt, c)

                yt_tiles = {}

                def emit_proj_m2(n, m2, split=False):
                    if n not in yt_tiles:
                        yt_tiles[n] = smp.tile([128, 3, 2, 512], BF16,
                                               tag="yT", bufs=2, name="yt_t")
                    yt_t = yt_tiles[n]
                    pp = ps.tile([128, 2, 512], F32, tag="sc", bufs=3,
                                 name="pp")
                    csplits = range(4) if split else (0,)
                    w = 512 // len(csplits)
                    for h in range(2):
                        m = 2 * m2 + h
                        for ci in csplits:
                            for kc in range(3):
                                nc.tensor.matmul(
                                    pp[:, h, ci * w : (ci + 1) * w],
                                    wp_r[:, kc, m * 128 : (m + 1) * 128],
                                    aoT[:, kc, n * 512 + ci * w
                                        : n * 512 + (ci + 1) * w],
                                    start=(kc == 0),
                                    stop=(kc == 2),
                                    skip_group_check=True,
                                )
                    if split or m2 % 2 == 0:
                        # tail blocks: all copies on ACT (idle after the
                        # final exps drain; DVE still owes the last norms)
                        nc.scalar.copy(yt_t[:, m2], pp[:])
                    else:
                        nc.vector.tensor_copy(yt_t[:, m2], pp[:])
                    if split:
                        # tail blocks: per-m2 DMA so output drains ASAP
                        nc.sync.dma_start(
                            yT_ap[:, 2 * m2 : 2 * m2 + 2,
                                  n * 512 : (n + 1) * 512],
                            yt_t[:, m2],
                        )
                    elif m2 == 2:
                        nc.sync.dma_start(
                            yT_ap[:, :, n * 512 : (n + 1) * 512],
                            yt_t[:].rearrange("p a b s -> p (a b) s"),
                        )

                def emit_proj_block(n, split=False):
                    for m2 in range(3):
                        emit_proj_m2(n, m2, split=split)

                def emit_proj_fillers(n):
                    return [lambda m2=m2: emit_proj_m2(n, m2)
                            for m2 in range(3)]

                prev = None
                proj_pending = []
                pre_pt = {}
                for p in range(PAIRS):
                    for n in range(NBLK):
                        if (p, n) in pre_pt:
                            pt = pre_pt.pop((p, n))
                        else:
                            pt = ptp.tile([128, 16, 2, 512], BF16,
                                          tag="pt", bufs=3,
                                          name=f"pt_{p}_{n}")
                        # interleave prev block's AV chunks into this
                        # block's score stream (safe once v is complete)
                        hook = None
                        if prev is not None and not (p == 0 and n <= 1):
                            pp_, pn_, ppt_ = prev
                            hook = lambda c: emit_av_chunk(pp_, pn_, ppt_, c)
                        if p == 0 and n == 0:
                            # k for the whole pair arrives per qk block;
                            # emit score units one qk block behind so the
                            # PE has qk work during each drain round-trip
                            emit_qk(0, 0)
                            pos = 0
                            for kb in range(NBLK):
                                if kb + 1 < NBLK:
                                    emit_qk(0, kb + 1)
                                for i in (2 * kb, 2 * kb + 1):
                                    for hi in range(2):
                                        # engine by EMISSION position: the
                                        # (i, hi)-interleaved order would
                                        # turn the u-indexed pattern into
                                        # 4-long same-engine runs
                                        emit_score_unit(
                                            0, 0, pt, hi, i,
                                            act=ACT_UNIT[0][pos])
                                        pos += 1
                            emit_qk(1, 0)
                        else:
                            work = []
                            if p == 0 and n == 1:
                                # merged stream: the v lump makes this
                                # block PE-bound, so pull the first half
                                # of block (0,2)'s score units forward
                                # (same q8/k8 pair) to feed the idle exp
                                # engines; (0,2) keeps the rest
                                pt02 = ptp.tile([128, 16, 2, 512], BF16,
                                                tag="pt", bufs=3,
                                                name="pt_0_2")
                                pre_pt[(0, 2)] = pt02
                                own = [(pt, 1, hi, i)
                                       for hi in range(2)
                                       for i in range(NPAIR)]
                                fwd = [(pt02, 2, hi, i)
                                       for hi in range(2)
                                       for i in range(5)]
                                seq = []
                                while own or fwd:
                                    seq += own[:4]
                                    own = own[4:]
                                    seq += fwd[:1]
                                    fwd = fwd[1:]
                                fillers = ([lambda ip=ip: emit_v(ip)
                                            for ip in range(NPAIR)]
                                           + emit_qk_fillers(1, 1))
                                for pos, (ptx, nn, hi, i) in enumerate(seq):
                                    emit_score_unit(
                                        0, nn, ptx, hi, i,
                                        act=ACT_UNIT[0][pos % 16])
                                    left = len(seq) - pos
                                    share = -(-len(fillers) // left)
                                    for _ in range(share):
                                        fillers.pop(0)()
                                for f in fillers:
                                    f()
                                emit_av(*prev)
                                prev = (p, n, pt)
                                continue
                            if p == 0 and n == 2:
                                # remaining half of this block's units
                                fillers = emit_qk_fillers(1, 2)
                                rem = [(hi, i) for hi in range(2)
                                       for i in range(5, NPAIR)]
                                for idx, (hi, i) in enumerate(rem):
                                    emit_score_unit(0, 2, pt, hi, i)
                                    left = len(rem) - idx
                                    share = -(-len(fillers) // left)
                                    for _ in range(share):
                                        fillers.pop(0)()
                                    if idx in (1, 2, 4, 5):
                                        hook((1, 2, 4, 5).index(idx))
                                for f in fillers:
                                    f()
                                prev = (p, n, pt)
                                continue
                            if p == PAIRS - 1 and proj_pending:
                                work.append(emit_proj_fillers(
                                    proj_pending.pop(0)))
                            fillers = []
                            k = 0
                            while any(work):
                                lst = work[k % len(work)]
                                if lst:
                                    fillers.append(lst.pop(0))
                                k += 1
                            if p < PAIRS - 1:
                                fillers += emit_qk_fillers(p + 1, n)
                            ha = ((3, 7, 10, 13)
                                  if (p, n) == (PAIRS - 1, NBLK - 1)
                                  else (3, 7, 11, 14))
                            emit_scores(p, n, pt, hook=hook,
                                        fillers=fillers, hook_at=ha)
                        if prev is not None and hook is None:
                            emit_av(*prev)
                        # pair-2 block n's AV lands via the hook during
                        # (2, n+1); its proj then fills block (2, n+2)
                        if prev is not None and prev[0] == PAIRS - 1:
                            proj_pending.append(prev[1])
                        prev = (p, n, pt)
                # tail: second-to-last proj first (chunk-split; its aoT
                # chunks finish during the last score block, and it keeps
                # the PE warm while the last exps drain), then the last
                # block's AV, then the final proj
                pp_, pn_, ppt_ = prev
                emit_proj_block(proj_pending.pop(0), split=True)
                emit_av(pp_, pn_, ppt_)
                emit_proj_block(pn_, split=True)

    nc.finalize()
    return nc


_NC_CACHE = {}


def _get_nc(S):
    if S not in _NC_CACHE:
        _NC_CACHE[S] = build_nc(S)
    return _NC_CACHE[S]


def kernel(x, qkv_w, qkv_b, proj_w, proj_b, return_res=False, **run_kwargs):
    import ml_dtypes

    x = np.asarray(x, dtype=np.float32)
    qkv_w = np.asarray(qkv_w, dtype=np.float32)
    qkv_b = np.asarray(qkv_b, dtype=np.float32)
    proj_w = np.asarray(proj_w, dtype=np.float32)
    proj_b = np.asarray(proj_b, dtype=np.float32)
    B, S, _ = x.shape

    nc = _get_nc(S)
    bf = ml_dtypes.bfloat16
    f8 = ml_dtypes.float8_e4m3
    in_maps = []
    for c in range(8):
        b, g = c // 2, c % 2
        qs = slice(384 * g, 384 * g + 384)
        ks = slice(768 + 384 * g, 768 + 384 * g + 384)
        vs = slice(1536 + 384 * g, 1536 + 384 * g + 384)
        SC = 64.0  # must match kernel WSCALE
        xTf = x[b].astype(bf).astype(np.float32).T  # [768, S]
        x8 = xTf.astype(f8)
        xr8 = (xTf - x8.astype(np.float32)).astype(f8)
        wqk = np.concatenate([qkv_w[:, qs], qkv_w[:, ks]], axis=1) * SC
        w8 = wqk.astype(f8)
        wr8 = (wqk - w8.astype(np.float32)).astype(f8)
        wvf = qkv_w[:, vs] * SC
        wv8 = wvf.astype(f8)
        wvr8 = (wvf - wv8.astype(np.float32)).astype(f8)
        in_maps.append(
            {
                "x8": np.ascontiguousarray(x8).view(np.uint8),
                "xr8": np.ascontiguousarray(xr8).view(np.uint8),
                "w8": np.ascontiguousarray(w8).view(np.uint8),
                "wr8": np.ascontiguousarray(wr8).view(np.uint8),
                "bqk": np.ascontiguousarray(
                    np.concatenate([qkv_b[qs], qkv_b[ks]]) * SC
                ),
                "wv8": np.ascontiguousarray(wv8).view(np.uint8),
                "wvr8": np.ascontiguousarray(wvr8).view(np.uint8),
                "wp": np.ascontiguousarray(
                    (proj_w[384 * g : 384 * g + 384, :] / SC).astype(bf)
                ).view(np.uint16),
            }
        )
    try:
        res = run_bass_kernel_spmd(
            nc, in_maps, core_ids=list(range(8)), **run_kwargs
        )
    except Exception:
        # transient NRT/device errors happen occasionally; retry once
        res = run_bass_kernel_spmd(
            nc, in_maps, core_ids=list(range(8)), **run_kwargs
        )
    # effective bias: the v bias passes through softmax additively
    b_eff = (proj_b.astype(np.float64)
             + qkv_b[1536:].astype(np.float64) @ proj_w.astype(np.float64)
             ).astype(np.float32)
    out = np.empty((B, S, HID), np.float32)
    for b in range(B):
        yt = (res.results[2 * b]["yT"].view(ml_dtypes.bfloat16)
              .astype(np.float32)
              + res.results[2 * b + 1]["yT"].view(ml_dtypes.bfloat16)
              .astype(np.float32))
        out[b] = yt.T + b_eff
    if return_res:
        return out, res
    return out
